# revision 34
# baseline (speedup 1.0000x reference)
"""StyleGAN2 conv_downsample_2d (FIR [1,3,3,1] + strided 1x1 conv) on 8 TRN2 cores.

Math (NCHW, per sample n):
    out[co, i, j] = sum_ci w[ci,co] * sum_{dy,dx} K2D[dy,dx] * x[ci, 2i+dy-1, 2j+dx-1]
with K2D = outer(k,k)/64, k = [1,3,3,1]  (symmetric, so the spatial flip is a no-op).

Decomposition per core (data-parallel over (sample, H-half) -> 8 shards):
  1. Vertical 4-tap FIR at row-stride 2 on VectorE, using the pair-sum form
     v = (x0+x3) + 3*(x1+x2)   (unnormalized; the /64 and the horizontal taps
     are folded into the 1x1-conv weights on the host).
  2. Horizontal FIR + channel mix fused on TensorE: 4 PSUM-accumulating
     matmuls per output tile, tap dx selected by a stride-2 column offset in
     the rhs access pattern; lhsT = w * k[dx]/64 (precomputed on host).
  3. PSUM -> SBUF on ScalarE, DMA out.

Each shard is host-padded to a uniform [128, 258, 512] row window so all 8
cores run the identical SPMD program (no partition-id branching). I/O is bf16
(memory-bound kernel; halves HBM traffic) and rows are host-packed in
polyphase column order so all DVE ops are step-1 16-bit (2x perf mode).
"""

import ml_dtypes
import numpy as np

import concourse.bass as bass
import concourse.mybir as mybir
from concourse import bacc
from concourse.tile import TileContext
from concourse.bass_utils import run_bass_kernel_spmd

N_CORES = 8
C_IN = 128
C_OUT = 256
H = 512
W = 512
HO = 256  # full output rows; 128 per core
WO = 256
SHARD_ROWS = 258  # 2*128 rows of taps + 2 boundary rows (host zero-padded)
TILE_ROWS = 16
N_TILES = 17  # 16 full 16-row tiles + one 2-row tail tile
N_CHUNKS = 16  # v-chunks of 8 output rows -> 128 output rows per core
VW = 516  # v row: [E 0..255 | zero 256,257 | O 258..513 | dead 514,515]

F32 = mybir.dt.float32
BF16 = mybir.dt.bfloat16

_CACHED_NC = None


def _build_program():
    nc = bacc.Bacc("TRN2", target_bir_lowering=False)

    # bf16 I/O halves HBM traffic vs f32 (this kernel is memory-bound);
    # accumulation stays f32/f32r on-chip, host up-casts the result.
    # x is tile-major and out is chunk-major so every DMA covers one fully
    # contiguous DRAM block (long HBM bursts, no strided runs); the host does
    # the cheap permutes.
    x = nc.dram_tensor("x", [N_TILES, C_IN, TILE_ROWS, W], BF16, kind="ExternalInput")
    wp = nc.dram_tensor("wp", [C_IN, 4, 2, 128], BF16, kind="ExternalInput")
    out = nc.dram_tensor("out", [N_CHUNKS, 128, 2, 8, WO], BF16, kind="ExternalOutput")

    with TileContext(nc) as tc:
        with (
            tc.tile_pool(name="inp", bufs=4) as inp_pool,
            tc.tile_pool(name="vpool", bufs=1) as v_pool,
            tc.tile_pool(name="stpool", bufs=1) as st_pool,
            tc.tile_pool(name="stage", bufs=2) as stage_pool,
            tc.tile_pool(name="wpool", bufs=1) as w_pool,
            tc.tile_pool(name="psum", bufs=2, space="PSUM") as psum_pool,
        ):
            wsb = w_pool.tile([C_IN, 4, 2, 128], BF16, tag="wr")
            nc.sync.dma_start(out=wsb[:], in_=wp[:])

            # s/t pair-sum scratch: fully rewritten every chunk (no carried
            # state -> no cross-chunk scheduling dependencies). bf16 in/out
            # with step-1 innermost keeps the DVE in its 2x perf mode.
            # Allocated as [.., 2, 256] so the E/O polyphase blocks are
            # addressable both flat (row ops) and per-block (v assembly).
            s = st_pool.tile([C_IN, 8, 2, 256], BF16, tag="s")
            t = st_pool.tile([C_IN, 8, 2, 256], BF16, tag="t")
            s3 = st_pool.tile([C_IN, 8, 2, 256], BF16, tag="s3")
            sf = s.rearrange("c m b k -> c m (b k)")
            tf = t.rearrange("c m b k -> c m (b k)")
            s3f = s3.rearrange("c m b k -> c m (b k)")

            # Two persistent v tiles (manual double-buffer) whose zero-pad
            # cells are written once up front instead of every chunk.
            vts = [
                v_pool.tile([C_IN, 8, 2, 258], BF16, tag=f"v{i}", name=f"v{i}")
                for i in range(2)
            ]
            vfs = [vt.rearrange("c m b k -> c m (b k)") for vt in vts]
            for vt in vts:
                nc.vector.memset(vt[:, :, 0, 256:258], 0.0)

            tiles: dict[int, object] = {}

            def in_tile(k):
                if k not in tiles:
                    t = inp_pool.tile([C_IN, TILE_ROWS, W], BF16, tag="in")
                    rows = 2 if k == N_TILES - 1 else TILE_ROWS
                    nc.sync.dma_start(
                        out=t[:, 0:rows, :],
                        in_=x[k, :, 0:rows, :],
                    )
                    tiles[k] = t
                return tiles[k]

            # rhs start column per horizontal tap dx into the flat v row
            # [E(256) | 0,0 | O(256)]: tap dx reads v_pad[2j+dx] where
            # v_pad = [0, v_row, 0]; even v_pad idx -> O block, odd -> E.
            TAP_OFF = [257, 0, 258, 1]



            def emit_block(vrow0, nrows, ta, tb, roff, v, vflat):
                """One v-block: v rows [vrow0, vrow0+nrows). Tap m (0..nrows)
                reads tile `ta` local rows roff+2m .. roff+2m+3, spilling into
                the first two rows of tile `tb` when past row 15."""
                # s[m] = x[2m+1] + x[2m+2]   (middle taps, weight 3)
                # t[m] = x[2m] + x[2m+3]     (outer taps, weight 1)
                ms = min(nrows, (13 - roff) // 2 + 1)  # rows with 2m+2+roff <= 15
                mt = min(nrows, (12 - roff) // 2 + 1)  # rows with 2m+3+roff <= 15
                nc.vector.tensor_add(
                    out=sf[:, 0:ms, :],
                    in0=ta[:, roff + 1 : roff + 2 * ms : 2, :],
                    in1=ta[:, roff + 2 : roff + 2 * ms + 1 : 2, :],
                )
                if ms < nrows:  # single boundary row: x[15] + next[0]
                    # boundary rows run on the otherwise-idle GpSimd engine,
                    # keeping the (near-critical) DVE queue shorter
                    nc.gpsimd.tensor_add(
                        out=sf[:, ms : ms + 1, :], in0=ta[:, 15:16, :], in1=tb[:, 0:1, :]
                    )
                nc.vector.tensor_add(
                    out=tf[:, 0:mt, :],
                    in0=ta[:, roff : roff + 2 * mt - 1 : 2, :],
                    in1=ta[:, roff + 3 : roff + 2 * mt + 2 : 2, :],
                )
                if mt < nrows:  # single boundary row: x[14] + next[1]
                    nc.gpsimd.tensor_add(
                        out=tf[:, mt : mt + 1, :], in0=ta[:, 14:15, :], in1=tb[:, 1:2, :]
                    )

                # v_row = 3*s + t in the host's polyphase column order. stt has
                # no 16-bit fast-mode uop, so build v as a 4x-mode
                # tensor_scalar (s3 = 3*s) plus one 2x-mode tensor_tensor add
                # whose output AP spans both E/O blocks around the
                # (pre-zeroed) pad cells.
                nc.vector.tensor_scalar_mul(s3f[:, 0:nrows, :], sf[:, 0:nrows, :], 3.0)
                nc.vector.tensor_add(
                    out=v[:, 0:nrows, :, 0:256],
                    in0=s3[:, 0:nrows, :, :],
                    in1=t[:, 0:nrows, :, :],
                )

                # Horizontal FIR + 1x1 conv: out[co, m, j] = sum_dx lhsT_dx.T @ v_pad[., 2j+dx]
                # Both co-halves land in one stage tile so the chunk's output
                # leaves in a single 2MB DMA (fewer HBM write turnarounds).
                stage = stage_pool.tile([128, 2, nrows, WO], BF16, tag="stage")
                for half in range(2):
                    # One multi-bank PSUM tile per half; 4-row accumulation
                    # groups (1024-col bf16 rhs) halve the per-matmul overhead
                    # vs 2-row groups. The whole tile drains with a single ACT
                    # copy (per-op bubble would dominate with per-bank copies).
                    p = psum_pool.tile([128, nrows, WO], F32, tag="ps")
                    for rp in range(nrows // 2):
                        for dx in range(4):
                            off = TAP_OFF[dx]
                            nc.tensor.matmul(
                                p[:, 2 * rp : 2 * rp + 2, :],
                                wsb[:, dx, half, :],
                                vflat[:, 2 * rp : 2 * rp + 2, off : off + 256],
                                start=(dx == 0),
                                stop=(dx == 3),
                            )
                    nc.scalar.copy(out=stage[:, half], in_=p[:])
                nc.sync.dma_start(
                    out=out[vrow0 // 8, :, :, vrow0 % 8 : vrow0 % 8 + nrows, :],
                    in_=stage[:],
                )

            for c in range(N_CHUNKS - 1):
                # v-chunk c needs shard rows 16c..16c+17: exactly tile c plus
                # the first two rows of tile c+1.
                emit_block(8 * c, 8, in_tile(c), in_tile(c + 1), 0, vts[c % 2], vfs[c % 2])
            # Split the final chunk into two 4-row blocks so its first half's
            # outputs stream out while the second half computes — shortens the
            # end-of-kernel drain after the input stream finishes.
            last = N_CHUNKS - 1
            emit_block(8 * last, 4, in_tile(last), None, 0, vts[last % 2], vfs[last % 2])
            emit_block(
                8 * last + 4, 4, in_tile(last), in_tile(last + 1), 8,
                vts[(last + 1) % 2], vfs[(last + 1) % 2],
            )
    nc.finalize()
    return nc


def _get_nc():
    global _CACHED_NC
    if _CACHED_NC is None:
        _CACHED_NC = _build_program()
    return _CACHED_NC


def _prep_inputs(images, w):
    images = np.asarray(images, dtype=np.float32)
    w = np.asarray(w, dtype=np.float32)
    assert images.shape == (4, C_IN, H, W), images.shape
    assert w.shape == (1, 1, C_IN, C_OUT), w.shape

    k = np.array([1.0, 3.0, 3.0, 1.0], dtype=np.float32)
    # wq[ci, dx, half, co] = w[ci, 128*half+co] * k[dx] / 64
    wq = np.ascontiguousarray(
        w[0, 0].reshape(C_IN, 1, 2, 128) * (k / 64.0).reshape(1, 4, 1, 1)
    ).astype(ml_dtypes.bfloat16)

    bf16 = ml_dtypes.bfloat16
    zrow = np.zeros((C_IN, 1, W), dtype=bf16)
    # Polyphase column packing: row -> [even cols (256) | odd cols (256)], so
    # every on-device DVE op and PE rhs slice is contiguous (16-bit fast mode).
    imgs16 = np.ascontiguousarray(
        images.astype(bf16).reshape(4, C_IN, H, W // 2, 2).transpose(0, 1, 2, 4, 3)
    ).reshape(4, C_IN, H, W)
    ztail = np.zeros((C_IN, N_TILES * TILE_ROWS - SHARD_ROWS, W), dtype=bf16)

    def to_tiles(shard):
        # [C, 258, W] -> tile-major [N_TILES, C, 16, W] (pad rows to 17*16)
        padded = np.concatenate([shard, ztail], axis=1)
        return np.ascontiguousarray(
            padded.reshape(C_IN, N_TILES, TILE_ROWS, W).transpose(1, 0, 2, 3)
        )

    in_maps = []
    for n in range(4):
        # half 0: padded global rows -1..256 ; half 1: padded global rows 255..512
        shard0 = to_tiles(np.concatenate([zrow, imgs16[n][:, 0:257, :]], axis=1))
        shard1 = to_tiles(np.concatenate([imgs16[n][:, 255:512, :], zrow], axis=1))
        in_maps.append({"x": shard0, "wp": wq})
        in_maps.append({"x": shard1, "wp": wq})
    return in_maps


def _assemble(results):
    out = np.empty((4, C_OUT, HO, WO), dtype=np.float32)
    for n in range(4):
        for half in range(2):
            # device out: [chunk, co_local, co_half, row, col] -> [C_OUT, 128, WO]
            buf = results[2 * n + half]["out"]
            res = buf.transpose(2, 1, 0, 3, 4).reshape(C_OUT, HO // 2, WO)
            out[n, :, 128 * half : 128 * (half + 1), :] = res
    return out


def run(images, w, **spmd_kwargs):
    """Full pipeline; returns (output, BassKernelResults)."""
    nc = _get_nc()
    in_maps = _prep_inputs(images, w)
    res = run_bass_kernel_spmd(nc, in_maps, core_ids=list(range(N_CORES)), **spmd_kwargs)
    return _assemble(res.results), res


def kernel(images, w):
    out, _ = run(images, w)
    return out



# revision 36
# speedup vs baseline: 1.2308x; 1.2308x over previous
"""StyleGAN2 conv_downsample_2d (FIR [1,3,3,1] + strided 1x1 conv) on 8 TRN2 cores.

Math (NCHW, per sample n):
    out[co, i, j] = sum_ci w[ci,co] * sum_{dy,dx} K2D[dy,dx] * x[ci, 2i+dy-1, 2j+dx-1]
with K2D = outer(k,k)/64, k = [1,3,3,1]  (symmetric, so the spatial flip is a no-op).

Decomposition per core (data-parallel over (sample, H-half) -> 8 shards):
  1. Vertical 4-tap FIR at row-stride 2 on VectorE, using the pair-sum form
     v = (x0+x3) + 3*(x1+x2)   (unnormalized; the /64 and the horizontal taps
     are folded into the 1x1-conv weights on the host).
  2. Horizontal FIR + channel mix fused on TensorE: 4 PSUM-accumulating
     matmuls per output tile, tap dx selected by a stride-2 column offset in
     the rhs access pattern; lhsT = w * k[dx]/64 (precomputed on host).
  3. PSUM -> SBUF on ScalarE, DMA out.

Each shard is host-padded to a uniform [128, 258, 512] row window so all 8
cores run the identical SPMD program (no partition-id branching). I/O is bf16
(memory-bound kernel; halves HBM traffic) and rows are host-packed in
polyphase column order so all DVE ops are step-1 16-bit (2x perf mode).
"""

import ml_dtypes
import numpy as np

import concourse.bass as bass
import concourse.mybir as mybir
from concourse import bacc
from concourse.tile import TileContext
from concourse.bass_utils import run_bass_kernel_spmd

N_CORES = 8
C_IN = 128
C_OUT = 256
H = 512
W = 512
HO = 256  # full output rows; 128 per core
WO = 256
SHARD_ROWS = 258  # 2*128 rows of taps + 2 boundary rows (host zero-padded)
TILE_ROWS = 16
N_TILES = 17  # 16 full 16-row tiles + one 2-row tail tile
N_CHUNKS = 16  # v-chunks of 8 output rows -> 128 output rows per core
VW = 516  # v row: [E 0..255 | zero 256,257 | O 258..513 | dead 514,515]

F32 = mybir.dt.float32
BF16 = mybir.dt.bfloat16

_CACHED_NC = None


def _build_program():
    nc = bacc.Bacc("TRN2", target_bir_lowering=False)

    # bf16 I/O halves HBM traffic vs f32 (this kernel is memory-bound);
    # accumulation stays f32/f32r on-chip, host up-casts the result.
    # x is tile-major and out is chunk-major so every DMA covers one fully
    # contiguous DRAM block (long HBM bursts, no strided runs); the host does
    # the cheap permutes.
    x = nc.dram_tensor("x", [N_TILES, C_IN, TILE_ROWS, W], BF16, kind="ExternalInput")
    wp = nc.dram_tensor("wp", [C_IN, 4, 2, 128], BF16, kind="ExternalInput")
    out = nc.dram_tensor("out", [N_CHUNKS, 128, 2, 8, WO], BF16, kind="ExternalOutput")

    with TileContext(nc) as tc:
        with (
            tc.tile_pool(name="inp", bufs=4) as inp_pool,
            tc.tile_pool(name="vpool", bufs=1) as v_pool,
            tc.tile_pool(name="stpool", bufs=1) as st_pool,
            tc.tile_pool(name="stage", bufs=2) as stage_pool,
            tc.tile_pool(name="wpool", bufs=1) as w_pool,
            tc.tile_pool(name="psum", bufs=2, space="PSUM") as psum_pool,
        ):
            wsb = w_pool.tile([C_IN, 4, 2, 128], BF16, tag="wr")
            nc.sync.dma_start(out=wsb[:], in_=wp[:])

            # s/t pair-sum scratch: fully rewritten every chunk (no carried
            # state -> no cross-chunk scheduling dependencies). bf16 in/out
            # with step-1 innermost keeps the DVE in its 2x perf mode.
            # Allocated as [.., 2, 256] so the E/O polyphase blocks are
            # addressable both flat (row ops) and per-block (v assembly).
            s = st_pool.tile([C_IN, 8, 2, 256], BF16, tag="s")
            t = st_pool.tile([C_IN, 8, 2, 256], BF16, tag="t")
            s3 = st_pool.tile([C_IN, 8, 2, 256], BF16, tag="s3")
            sf = s.rearrange("c m b k -> c m (b k)")
            tf = t.rearrange("c m b k -> c m (b k)")
            s3f = s3.rearrange("c m b k -> c m (b k)")

            # Two persistent v tiles (manual double-buffer) whose zero-pad
            # cells are written once up front instead of every chunk.
            vts = [
                v_pool.tile([C_IN, 8, 2, 258], BF16, tag=f"v{i}", name=f"v{i}")
                for i in range(2)
            ]
            vfs = [vt.rearrange("c m b k -> c m (b k)") for vt in vts]
            for vt in vts:
                nc.vector.memset(vt[:, :, 0, 256:258], 0.0)

            tiles: dict[int, object] = {}

            def in_tile(k):
                if k not in tiles:
                    t = inp_pool.tile([C_IN, TILE_ROWS, W], BF16, tag="in")
                    rows = 2 if k == N_TILES - 1 else TILE_ROWS
                    nc.sync.dma_start(
                        out=t[:, 0:rows, :],
                        in_=x[k, :, 0:rows, :],
                    )
                    tiles[k] = t
                return tiles[k]

            # rhs start column per horizontal tap dx into the flat v row
            # [E(256) | 0,0 | O(256)]: tap dx reads v_pad[2j+dx] where
            # v_pad = [0, v_row, 0]; even v_pad idx -> O block, odd -> E.
            TAP_OFF = [257, 0, 258, 1]



            def emit_block(vrow0, nrows, ta, tb, roff, v, vflat):
                """One v-block: v rows [vrow0, vrow0+nrows). Tap m (0..nrows)
                reads tile `ta` local rows roff+2m .. roff+2m+3, spilling into
                the first two rows of tile `tb` when past row 15."""
                # s[m] = x[2m+1] + x[2m+2]   (middle taps, weight 3)
                # t[m] = x[2m] + x[2m+3]     (outer taps, weight 1)
                ms = min(nrows, (13 - roff) // 2 + 1)  # rows with 2m+2+roff <= 15
                mt = min(nrows, (12 - roff) // 2 + 1)  # rows with 2m+3+roff <= 15
                nc.vector.tensor_add(
                    out=sf[:, 0:ms, :],
                    in0=ta[:, roff + 1 : roff + 2 * ms : 2, :],
                    in1=ta[:, roff + 2 : roff + 2 * ms + 1 : 2, :],
                )
                if ms < nrows:  # single boundary row: x[15] + next[0]
                    nc.vector.tensor_add(
                        out=sf[:, ms : ms + 1, :], in0=ta[:, 15:16, :], in1=tb[:, 0:1, :]
                    )
                nc.vector.tensor_add(
                    out=tf[:, 0:mt, :],
                    in0=ta[:, roff : roff + 2 * mt - 1 : 2, :],
                    in1=ta[:, roff + 3 : roff + 2 * mt + 2 : 2, :],
                )
                if mt < nrows:  # single boundary row: x[14] + next[1]
                    nc.vector.tensor_add(
                        out=tf[:, mt : mt + 1, :], in0=ta[:, 14:15, :], in1=tb[:, 1:2, :]
                    )

                # v_row = 3*s + t in the host's polyphase column order. stt has
                # no 16-bit fast-mode uop, so build v as a 4x-mode
                # tensor_scalar (s3 = 3*s) plus one 2x-mode tensor_tensor add
                # whose output AP spans both E/O blocks around the
                # (pre-zeroed) pad cells.
                nc.vector.tensor_scalar_mul(s3f[:, 0:nrows, :], sf[:, 0:nrows, :], 3.0)
                nc.vector.tensor_add(
                    out=v[:, 0:nrows, :, 0:256],
                    in0=s3[:, 0:nrows, :, :],
                    in1=t[:, 0:nrows, :, :],
                )

                # Horizontal FIR + 1x1 conv: out[co, m, j] = sum_dx lhsT_dx.T @ v_pad[., 2j+dx]
                # Both co-halves land in one stage tile so the chunk's output
                # leaves in a single 2MB DMA (fewer HBM write turnarounds).
                stage = stage_pool.tile([128, 2, nrows, WO], BF16, tag="stage")
                for half in range(2):
                    # One multi-bank PSUM tile per half; 4-row accumulation
                    # groups (1024-col bf16 rhs) halve the per-matmul overhead
                    # vs 2-row groups. The whole tile drains with a single ACT
                    # copy (per-op bubble would dominate with per-bank copies).
                    p = psum_pool.tile([128, nrows, WO], F32, tag="ps")
                    for rp in range(nrows // 2):
                        for dx in range(4):
                            off = TAP_OFF[dx]
                            nc.tensor.matmul(
                                p[:, 2 * rp : 2 * rp + 2, :],
                                wsb[:, dx, half, :],
                                vflat[:, 2 * rp : 2 * rp + 2, off : off + 256],
                                start=(dx == 0),
                                stop=(dx == 3),
                            )
                    nc.scalar.copy(out=stage[:, half], in_=p[:])
                nc.sync.dma_start(
                    out=out[vrow0 // 8, :, :, vrow0 % 8 : vrow0 % 8 + nrows, :],
                    in_=stage[:],
                )

            for c in range(N_CHUNKS - 1):
                # v-chunk c needs shard rows 16c..16c+17: exactly tile c plus
                # the first two rows of tile c+1.
                emit_block(8 * c, 8, in_tile(c), in_tile(c + 1), 0, vts[c % 2], vfs[c % 2])
            # Split the final chunk into two 4-row blocks so its first half's
            # outputs stream out while the second half computes — shortens the
            # end-of-kernel drain after the input stream finishes.
            last = N_CHUNKS - 1
            emit_block(8 * last, 4, in_tile(last), None, 0, vts[last % 2], vfs[last % 2])
            emit_block(
                8 * last + 4, 4, in_tile(last), in_tile(last + 1), 8,
                vts[(last + 1) % 2], vfs[(last + 1) % 2],
            )
    nc.finalize()
    return nc


def _get_nc():
    global _CACHED_NC
    if _CACHED_NC is None:
        _CACHED_NC = _build_program()
    return _CACHED_NC


def _prep_inputs(images, w):
    images = np.asarray(images, dtype=np.float32)
    w = np.asarray(w, dtype=np.float32)
    assert images.shape == (4, C_IN, H, W), images.shape
    assert w.shape == (1, 1, C_IN, C_OUT), w.shape

    k = np.array([1.0, 3.0, 3.0, 1.0], dtype=np.float32)
    # wq[ci, dx, half, co] = w[ci, 128*half+co] * k[dx] / 64
    wq = np.ascontiguousarray(
        w[0, 0].reshape(C_IN, 1, 2, 128) * (k / 64.0).reshape(1, 4, 1, 1)
    ).astype(ml_dtypes.bfloat16)

    bf16 = ml_dtypes.bfloat16
    zrow = np.zeros((C_IN, 1, W), dtype=bf16)
    # Polyphase column packing: row -> [even cols (256) | odd cols (256)], so
    # every on-device DVE op and PE rhs slice is contiguous (16-bit fast mode).
    imgs16 = np.ascontiguousarray(
        images.astype(bf16).reshape(4, C_IN, H, W // 2, 2).transpose(0, 1, 2, 4, 3)
    ).reshape(4, C_IN, H, W)
    ztail = np.zeros((C_IN, N_TILES * TILE_ROWS - SHARD_ROWS, W), dtype=bf16)

    def to_tiles(shard):
        # [C, 258, W] -> tile-major [N_TILES, C, 16, W] (pad rows to 17*16)
        padded = np.concatenate([shard, ztail], axis=1)
        return np.ascontiguousarray(
            padded.reshape(C_IN, N_TILES, TILE_ROWS, W).transpose(1, 0, 2, 3)
        )

    in_maps = []
    for n in range(4):
        # half 0: padded global rows -1..256 ; half 1: padded global rows 255..512
        shard0 = to_tiles(np.concatenate([zrow, imgs16[n][:, 0:257, :]], axis=1))
        shard1 = to_tiles(np.concatenate([imgs16[n][:, 255:512, :], zrow], axis=1))
        in_maps.append({"x": shard0, "wp": wq})
        in_maps.append({"x": shard1, "wp": wq})
    return in_maps


def _assemble(results):
    out = np.empty((4, C_OUT, HO, WO), dtype=np.float32)
    for n in range(4):
        for half in range(2):
            # device out: [chunk, co_local, co_half, row, col] -> [C_OUT, 128, WO]
            buf = results[2 * n + half]["out"]
            res = buf.transpose(2, 1, 0, 3, 4).reshape(C_OUT, HO // 2, WO)
            out[n, :, 128 * half : 128 * (half + 1), :] = res
    return out


def run(images, w, **spmd_kwargs):
    """Full pipeline; returns (output, BassKernelResults)."""
    nc = _get_nc()
    in_maps = _prep_inputs(images, w)
    res = run_bass_kernel_spmd(nc, in_maps, core_ids=list(range(N_CORES)), **spmd_kwargs)
    return _assemble(res.results), res


def kernel(images, w):
    out, _ = run(images, w)
    return out

